# revision 1
# baseline (speedup 1.0000x reference)
"""ContextAwareAttention Trainium2 kernel.

Strategy:
  - Data-parallel over batch: B=128 split as 16 batches per NeuronCore x 8 cores.
  - Host-side prep: transpose per-batch activations to feature-major, gather the
    relative-position bias table to [H, m, n], convert mask to multiplicative f32,
    fold the attention scale into Wq and (bo + bc2) into one combined bias.
  - Device kernel is fully feature-major so no on-device transposes are needed:
      Q^T = Wq'^T  @ x^T          [512, n]   (per batch)
      K^T = Wk^T   @ c^T          [512, n]
      V   = c @ Wv (token-major)  [m, 512]
      S^T = k_h^T.T @ q_h^T       [m, n] per head (head pairs run concurrently
                                   in disjoint 64-row groups of the PE array)
      P^T = exp(S^T) * exp(biasT) * maskT  (host pre-exponentiates the bias;
                                   no max-subtraction needed: logits are O(1))
      denom = ones[128].T @ [P_h0 | P_h1]  -> one [128, 512] PSUM tile with both
                                   heads' sums broadcast to all partitions
      O^T  = v_h.T @ P^T / denom
      out1^T = Wo^T @ O^T
      context branch feature-major with LN stats via ones-matmuls
      result^T = out1^T + ctx2^T (+ folded biases)
  - All matmuls use float32r (full PE rate at free-dim >= 256).
  - Output is written feature-major; the host transposes back.
"""

import numpy as np

import concourse.bass as bass  # noqa: F401  (engine types come via bacc)
import concourse.mybir as mybir
import concourse.tile as tile
from concourse import bacc
from concourse.bass_utils import run_bass_kernel_spmd

B, N, DIM, H, D = 128, 256, 512, 8, 64
N_CORES = 8
BPC = B // N_CORES            # batches per core = 16
SBATCH = 2                    # batches per superbatch
NSUPER = BPC // SBATCH        # 8
SCALE = D ** -0.5
LN_EPS = 1e-5
F32 = mybir.dt.float32
F32R = mybir.dt.float32r
NW = SBATCH * N               # free width of 2-batch tiles = 512

AF = mybir.ActivationFunctionType


def _emit(nc, tc, io, n_super):
    xT, cT, mT, bT, wq, wk, wv, wc1, wc2, wo, lng, lnb, bc1, bocb, onesd, outT = io

    def mm(out, lhsT, rhs, start, stop, tile_position=None):
        nc.tensor.matmul(out, lhsT, rhs, start=start, stop=stop,
                         tile_position=tile_position)

    with (
        nc.allow_low_precision(reason="fp32r-typed matmul operand tiles"),
        tc.tile_pool(name="consts", bufs=1) as consts,
        tc.tile_pool(name="io", bufs=2) as iop,
        tc.tile_pool(name="work", bufs=1) as work,
        tc.tile_pool(name="pp", bufs=6) as ppool,
        tc.tile_pool(name="rows", bufs=4) as rows,
        tc.tile_pool(name="psum", bufs=8, space="PSUM") as psum,
    ):
        # ---- constants ----
        ones = consts.tile([128, 512], F32R)
        nc.sync.dma_start(
            out=ones, in_=onesd.unsqueeze(0).partition_broadcast(128).squeeze(1))
        eps_sb = consts.tile([1, 1], F32, name="eps")
        nc.vector.memset(eps_sb, LN_EPS)
        w_sb = {}
        for nm, src in (("wq", wq),):
            t = consts.tile([128, 4, 512], F32R, name=f"w_{nm}")
            nc.sync.dma_start(out=t, in_=src.rearrange("(kc p) f -> p kc f", p=128))
            w_sb[nm] = t
        for nm, src in (("wk", wk), ("wv", wv), ("wc1", wc1), ("wc2", wc2),
                        ("wo", wo)):
            t = consts.tile([128, 4, 512], F32R, name=f"w_{nm}")
            nc.scalar.dma_start(out=t,
                                in_=src.rearrange("(kc p) f -> p kc f", p=128))
            w_sb[nm] = t
        bias_sb = consts.tile([128, 8, 2, 256], F32, name="biasT")
        nc.scalar.dma_start(out=bias_sb,
                            in_=bT.rearrange("h (mc p) n -> p h mc n", p=128))
        lng_sb = consts.tile([128, 4], F32, name="lng")
        nc.scalar.dma_start(out=lng_sb, in_=lng.rearrange("(c p) -> p c", p=128))
        lnb_sb = consts.tile([128, 4], F32, name="lnb")
        nc.scalar.dma_start(out=lnb_sb, in_=lnb.rearrange("(c p) -> p c", p=128))
        bc1_sb = consts.tile([1, 512], F32R, name="bc1r")
        nc.scalar.dma_start(out=bc1_sb, in_=bc1.unsqueeze(0))
        bocb_sb = consts.tile([1, 512], F32R, name="bocbr")
        nc.scalar.dma_start(out=bocb_sb, in_=bocb.unsqueeze(0))

        for sb in range(n_super):
            b0 = sb * SBATCH
            # ---- input DMA ----
            xt = iop.tile([128, 4, SBATCH, 256], F32R, name="xt")
            ct = iop.tile([128, 4, SBATCH, 256], F32R, name="ct")
            mt = iop.tile([128, 2, SBATCH, 256], F32, name="mt")
            for j in range(SBATCH):
                nc.sync.dma_start(
                    out=xt[:, :, j, :], in_=xT[b0 + j].rearrange(
                        "(kc p) n -> p kc n", p=128))
                nc.sync.dma_start(
                    out=ct[:, :, j, :], in_=cT[b0 + j].rearrange(
                        "(kc p) n -> p kc n", p=128))
                nc.sync.dma_start(
                    out=mt[:, :, j, :], in_=mT[b0 + j].rearrange(
                        "(mc p) n -> p mc n", p=128))

            # ---- projections ----
            qt = work.tile([128, 4, SBATCH, 256], F32R, name="qt")
            kt = work.tile([128, 4, SBATCH, 256], F32R, name="kt")
            for c in range(4):
                ps = psum.tile([128, NW], F32, tag="u")
                for kc in range(4):
                    mm(ps, w_sb["wq"][:, kc, c * 128:(c + 1) * 128],
                       xt[:, kc].rearrange("p j n -> p (j n)"),
                       start=kc == 0, stop=kc == 3)
                nc.scalar.copy(out=qt[:, c].rearrange("p j n -> p (j n)"), in_=ps)
            for c in range(4):
                ps = psum.tile([128, NW], F32, tag="u")
                for kc in range(4):
                    mm(ps, w_sb["wk"][:, kc, c * 128:(c + 1) * 128],
                       ct[:, kc].rearrange("p j n -> p (j n)"),
                       start=kc == 0, stop=kc == 3)
                nc.scalar.copy(out=kt[:, c].rearrange("p j n -> p (j n)"), in_=ps)
            vt = work.tile([128, 2, SBATCH, 512], F32R, name="vt")
            for j in range(SBATCH):
                for mc in range(2):
                    ps = psum.tile([128, 512], F32, tag="u")
                    for kc in range(4):
                        mm(ps, ct[:, kc, j, mc * 128:(mc + 1) * 128],
                           w_sb["wv"][:, kc], start=kc == 0, stop=kc == 3)
                    nc.scalar.copy(out=vt[:, mc, j], in_=ps)

            # ---- context branch: h = c @ Wc1 + bc1 (feature-major) ----
            ht = work.tile([128, 4, NW], F32R, name="ht")
            for c in range(4):
                ps = psum.tile([128, NW], F32, tag="u")
                for kc in range(4):
                    mm(ps, w_sb["wc1"][:, kc, c * 128:(c + 1) * 128],
                       ct[:, kc].rearrange("p j n -> p (j n)"),
                       start=kc == 0, stop=False)
                mm(ps, bc1_sb[0:1, c * 128:(c + 1) * 128], ones[0:1, :],
                   start=False, stop=True)
                nc.scalar.copy(out=ht[:, c], in_=ps)

            # LN stats: column sums of h and h^2 via ones-matmuls
            sqt = work.tile([128, NW], F32R, name="sqt", bufs=2)
            mu_ps = psum.tile([1, NW], F32, tag="u")
            sq_ps = psum.tile([1, NW], F32, tag="u")
            for c in range(4):
                mm(mu_ps, ones[:, 0:1], ht[:, c], start=c == 0, stop=c == 3)
            for c in range(4):
                nc.gpsimd.tensor_mul(out=sqt, in0=ht[:, c].bitcast(F32),
                                     in1=ht[:, c].bitcast(F32))
                mm(sq_ps, ones[:, 0:1], sqt, start=c == 0, stop=c == 3)
            m_r = rows.tile([1, NW], F32, tag="r")
            nc.scalar.mul(out=m_r, in_=mu_ps, mul=1.0 / DIM)
            q_r = rows.tile([1, NW], F32, tag="r")
            nc.scalar.mul(out=q_r, in_=sq_ps, mul=1.0 / DIM)
            ms_r = rows.tile([1, NW], F32, tag="r")
            nc.vector.tensor_mul(out=ms_r, in0=m_r, in1=m_r)
            var_r = rows.tile([1, NW], F32, tag="r")
            nc.vector.tensor_sub(out=var_r, in0=q_r, in1=ms_r)
            sd_r = rows.tile([1, NW], F32R, tag="r")
            nc.scalar.activation(out=sd_r, in_=var_r, func=AF.Sqrt, bias=eps_sb)
            a_r = rows.tile([1, NW], F32R, tag="r")
            nc.vector.reciprocal(out=a_r, in_=sd_r)
            d_r = rows.tile([1, NW], F32R, tag="r")
            nc.vector.scalar_tensor_tensor(
                out=d_r, in0=m_r, scalar=-1.0, in1=a_r.bitcast(F32),
                op0=mybir.AluOpType.mult, op1=mybir.AluOpType.mult)
            # ---- attention ----
            ot = work.tile([128, 4, SBATCH, 256], F32R, name="ot")

            def softmax_stage(c, j):
                """scores + P for head pair c, batch j. One P tile
                [128, mc, h2, n] computed in place: exp(S) * exp(bias) * mask."""
                pt = ppool.tile([128, 2, 2, 256], F32R, tag="p", name="pt",
                                bufs=3)
                for h2 in range(2):
                    p0 = 64 * h2
                    s_ps = psum.tile([128, 512], F32, tag="u", name="sps")
                    for mc in range(2):
                        mm(s_ps[:, mc * 256:(mc + 1) * 256],
                           kt[p0:p0 + 64, c, j, mc * 128:(mc + 1) * 128],
                           qt[p0:p0 + 64, c, j], start=True, stop=True)
                    dst = pt[:, :, h2, :]
                    nc.scalar.activation(
                        out=dst, in_=s_ps.rearrange("p (m n) -> p m n", m=2),
                        func=AF.Exp)
                    nc.vector.tensor_mul(out=dst, in0=dst.bitcast(F32),
                                         in1=bias_sb[:, 2 * c + h2])
                    nc.gpsimd.tensor_mul(out=dst, in0=dst.bitcast(F32),
                                         in1=mt[:, :, j, :])
                return pt

            def reduce_stage(c, j, pt):
                """denominators + attn@v + normalize into ot."""
                sbc = psum.tile([128, 512], F32, tag="u")
                for mc in range(2):
                    mm(sbc, ones[:, 0:128],
                       pt[:, mc].rearrange("p h n -> p (h n)"),
                       start=mc == 0, stop=mc == 1)
                oo = {}
                for h2 in range(2):
                    hd = (2 * c + h2) * 64
                    oo[h2] = psum.tile([64, 256], F32, tag="u", name="oo")
                    for mc in range(2):
                        mm(oo[h2], vt[:, mc, j, hd:hd + 64],
                           pt[:, mc, h2, :], start=mc == 0, stop=mc == 1)
                rec = ppool.tile([128, 512], F32, tag="rec", bufs=2)
                nc.vector.reciprocal(out=rec, in_=sbc)
                for h2 in range(2):
                    nc.vector.tensor_mul(
                        out=ot[h2 * 64:(h2 + 1) * 64, c, j], in0=oo[h2],
                        in1=rec[h2 * 64:(h2 + 1) * 64,
                                h2 * 256:(h2 + 1) * 256])

            def ln_normalize():
                # broadcast a (rstd) and d (-mu*rstd) to 128 partitions
                a_bc = psum.tile([128, NW], F32, tag="u")
                mm(a_bc, ones[0:1, 0:128], a_r, start=True, stop=True)
                d_bc = psum.tile([128, NW], F32, tag="u")
                mm(d_bc, ones[0:1, 0:128], d_r, start=True, stop=True)
                # normalized = (h*a + d) * g + b, then relu  (g,b per-partition)
                for c in range(4):
                    t1 = work.tile([128, NW], F32, name="t1", bufs=2)
                    nc.vector.scalar_tensor_tensor(
                        out=t1, in0=ht[:, c].bitcast(F32),
                        scalar=lng_sb[:, c:c + 1], in1=a_bc,
                        op0=mybir.AluOpType.mult, op1=mybir.AluOpType.mult)
                    nc.vector.scalar_tensor_tensor(
                        out=ht[:, c], in0=d_bc, scalar=lng_sb[:, c:c + 1], in1=t1,
                        op0=mybir.AluOpType.mult, op1=mybir.AluOpType.add)
                    nc.scalar.activation(out=ht[:, c], in_=ht[:, c].bitcast(F32), func=AF.Relu,
                                         bias=lnb_sb[:, c:c + 1], scale=1.0)

            ctx2 = work.tile([128, 4, NW], F32, name="ctx2")

            def ctx2_chunk(c):
                ps = psum.tile([128, NW], F32, tag="u", name="cps")
                for kc in range(4):
                    mm(ps, w_sb["wc2"][:, kc, c * 128:(c + 1) * 128], ht[:, kc],
                       start=kc == 0, stop=False)
                mm(ps, bocb_sb[0:1, c * 128:(c + 1) * 128], ones[0:1, :],
                   start=False, stop=True)
                nc.scalar.copy(out=ctx2[:, c], in_=ps)

            res = iop.tile([128, 4, SBATCH, 256], F32, name="res")

            def out_proj(j):
                for c in range(4):
                    ps = psum.tile([128, 256], F32, tag="u", name="ops")
                    for kc in range(4):
                        mm(ps, w_sb["wo"][:, kc, c * 128:(c + 1) * 128],
                           ot[:, kc, j], start=kc == 0, stop=kc == 3)
                    nc.vector.tensor_add(out=res[:, c, j], in0=ps,
                                         in1=ctx2[:, c, j * 256:(j + 1) * 256])

            pend = []
            iu = 0
            for j in range(SBATCH):
                for c in range(4):
                    pend.append((c, j, softmax_stage(c, j)))
                    if len(pend) > 2:
                        reduce_stage(*pend.pop(0))
                    iu += 1
                    if iu == 3:
                        ln_normalize()
                    if 4 <= iu <= 7:
                        ctx2_chunk(iu - 4)
                    if iu == 8:
                        out_proj(0)
            for u in pend:
                reduce_stage(*u)
            out_proj(1)

            for j in range(SBATCH):
                nc.gpsimd.dma_start(
                    out=outT[b0 + j].rearrange("(c p) n -> p c n", p=128),
                    in_=res[:, :, j, :])


def build(n_super=NSUPER):
    nc = bacc.Bacc("TRN2", target_bir_lowering=False, debug=False,
                   num_devices=N_CORES)
    dt = nc.dram_tensor
    io = (
        dt("xT", [BPC, DIM, N], F32R, kind="ExternalInput").ap(),
        dt("cT", [BPC, DIM, N], F32R, kind="ExternalInput").ap(),
        dt("mT", [BPC, N, N], F32, kind="ExternalInput").ap(),
        dt("bT", [H, N, N], F32, kind="ExternalInput").ap(),
        dt("wq", [DIM, DIM], F32R, kind="ExternalInput").ap(),
        dt("wk", [DIM, DIM], F32R, kind="ExternalInput").ap(),
        dt("wv", [DIM, DIM], F32R, kind="ExternalInput").ap(),
        dt("wc1", [DIM, DIM], F32R, kind="ExternalInput").ap(),
        dt("wc2", [DIM, DIM], F32R, kind="ExternalInput").ap(),
        dt("wo", [DIM, DIM], F32R, kind="ExternalInput").ap(),
        dt("lng", [DIM], F32, kind="ExternalInput").ap(),
        dt("lnb", [DIM], F32, kind="ExternalInput").ap(),
        dt("bc1", [DIM], F32R, kind="ExternalInput").ap(),
        dt("bocb", [DIM], F32R, kind="ExternalInput").ap(),
        dt("onesd", [DIM], F32R, kind="ExternalInput").ap(),
        dt("outT", [BPC, DIM, N], F32, kind="ExternalOutput").ap(),
    )
    with tile.TileContext(nc) as tc:
        _emit(nc, tc, io, n_super)
    nc.compile()
    return nc


def prep_in_maps(x, context, mask, Wq, Wk, Wv, Wc1, bc1, ln_g, ln_b, Wc2, bc2,
                 Wo, bo, bias_table, rel_index):
    f = np.float32
    xT = np.ascontiguousarray(
        x.reshape(N_CORES, BPC, N, DIM).transpose(0, 1, 3, 2)).astype(f, copy=False)
    cT = np.ascontiguousarray(
        context.reshape(N_CORES, BPC, N, DIM).transpose(0, 1, 3, 2)).astype(
            f, copy=False)
    mT = np.ascontiguousarray(
        mask.reshape(N_CORES, BPC, N, N).transpose(0, 1, 3, 2)).astype(f)
    bT = np.ascontiguousarray(np.exp(
        np.asarray(bias_table)[np.asarray(rel_index)].transpose(2, 1, 0))).astype(
            f, copy=False)
    shared = dict(
        bT=bT,
        wq=np.ascontiguousarray(Wq * SCALE).astype(f, copy=False),
        wk=np.ascontiguousarray(Wk).astype(f, copy=False),
        wv=np.ascontiguousarray(Wv).astype(f, copy=False),
        wc1=np.ascontiguousarray(Wc1).astype(f, copy=False),
        wc2=np.ascontiguousarray(Wc2).astype(f, copy=False),
        wo=np.ascontiguousarray(Wo).astype(f, copy=False),
        lng=np.ascontiguousarray(ln_g).astype(f, copy=False),
        lnb=np.ascontiguousarray(ln_b).astype(f, copy=False),
        bc1=np.ascontiguousarray(bc1).astype(f, copy=False),
        bocb=np.ascontiguousarray(bo + bc2).astype(f, copy=False),
        onesd=np.ones(DIM, dtype=f),
    )
    return [dict(xT=xT[c], cT=cT[c], mT=mT[c], **shared) for c in range(N_CORES)]


_nc_cache = {}


def _get_nc(n_super=NSUPER):
    if n_super not in _nc_cache:
        _nc_cache[n_super] = build(n_super)
    return _nc_cache[n_super]


def assemble_out(results):
    outT = np.stack([results[c]["outT"] for c in range(N_CORES)])  # [8,16,512,256]
    return np.ascontiguousarray(
        outT.transpose(0, 1, 3, 2).reshape(B, N, DIM)).astype(np.float32)


def kernel(**inputs):
    nc = _get_nc()
    in_maps = prep_in_maps(**inputs)
    res = run_bass_kernel_spmd(nc, in_maps, core_ids=list(range(N_CORES)))
    return assemble_out(res.results)



# revision 12
# speedup vs baseline: 1.1611x; 1.1611x over previous
"""ContextAwareAttention Trainium2 kernel (v2).

Strategy (sized for the TimelineSim cost model):
  - Data-parallel over batch: B=128 -> 16 batches/core x 8 cores; SBATCH=2
    batches per superbatch ("SB") iteration.
  - fp8e4m3 DoubleRow matmuls (0.5 cyc/row, 2x contraction per instruction)
    for the q/k/v projections and Wo: 4x fewer PE cycles than fp32r.
    Weights are pre-scaled by powers of two into fp8 range; scales cancel
    exactly (exp scale / V-ones column value / output copy scale).
  - bf16 on the element-wise engines (DVE 2x perf modes).
  - Softmax denominator rides the attention@V matmul as a 65th V column;
    reciprocal on a [1,512] row; broadcast back by one PE matmul.
  - mask and exp(rel-pos-bias) are pre-multiplied on the host into one bf16
    [b, head-pair, m, n] table -> single fused P multiply per (c,j).
  - LayerNorm rstd = exp(-0.5*ln(512*var+512*eps)): Ln/Exp/Relu/Square share
    one activation table set -> zero act-table reloads.
  - ctx2 (Wc2) and out1 (Wo) accumulate into one PSUM tile per (chunk, j);
    one copy applies the 2^-g rescale plus the combined bias.
"""

import math

import numpy as np
import ml_dtypes

import concourse.bass as bass  # noqa: F401
import concourse.mybir as mybir
import concourse.tile as tile
from concourse import bacc
from concourse.bass_utils import run_bass_kernel_spmd

B, N, DIM, H, D = 128, 256, 512, 8, 64
N_CORES = 8
BPC = B // N_CORES            # 16
SBATCH = 2
NSUPER = BPC // SBATCH        # 8
SCALE = D ** -0.5
LN_EPS = 1e-5
F32 = mybir.dt.float32
F32R = mybir.dt.float32r
BF16 = mybir.dt.bfloat16
F8 = mybir.dt.float8e4
NW = SBATCH * N               # 512
NP8 = np.dtype(ml_dtypes.float8_e4m3)
NPBF = np.dtype(ml_dtypes.bfloat16)

AF = mybir.ActivationFunctionType
ALU = mybir.AluOpType
DR = mybir.MatmulPerfMode.DoubleRow

OT_K = 5   # ot is stored as 2^OT_K * O/denom (fp8 range health)


def _emit(nc, tc, io, n_super, ks):
    (x8d, c8d, cbd, mbd, wq, wk, wv, wo, wc1, wc2, lngd, lnbd,
     bc1d, bocbd, outT) = io
    kq, kk, kv, kwo = ks
    g = OT_K + kwo

    def mm(out, lhsT, rhs, start, stop, perf_mode=None):
        nc.tensor.matmul(out, lhsT, rhs, start=start, stop=stop,
                         perf_mode=perf_mode)

    with (
        nc.allow_low_precision(reason="fp8/bf16 design, verified vs oracle"),
        tc.tile_pool(name="consts", bufs=1) as consts,
        tc.tile_pool(name="io", bufs=2) as iop,
        tc.tile_pool(name="mb", bufs=3) as mbp,
        tc.tile_pool(name="work", bufs=1) as work,
        tc.tile_pool(name="pp", bufs=3) as ppool,
        tc.tile_pool(name="rows", bufs=2) as rows,
        tc.tile_pool(name="psum", bufs=2, space="PSUM") as psum,
    ):
        # ---- compile-time constants (no DMA) ----
        onecol = consts.tile([1, 128], BF16, name="onecol")
        nc.vector.memset(onecol, 1.0)
        colones = consts.tile([128, 1], BF16, name="colones")
        nc.vector.memset(colones, 1.0)
        eps512 = consts.tile([1, 1], F32, name="eps512")
        nc.vector.memset(eps512, DIM * LN_EPS)

        # ---- DMA'd constants ----
        w8 = {}
        for nm, src in (("wq", wq), ("wk", wk), ("wv", wv), ("wo", wo)):
            t = consts.tile([128, 4, 512], F8, name=f"w_{nm}")
            nc.scalar.dma_start(out=t,
                                in_=src.rearrange("(kc p) f -> p kc f", p=128))
            w8[nm] = t
        wb = {}
        for nm, src in (("wc1", wc1), ("wc2", wc2)):
            t = consts.tile([128, 4, 512], BF16, name=f"w_{nm}")
            nc.scalar.dma_start(out=t,
                                in_=src.rearrange("(kc p) f -> p kc f", p=128))
            wb[nm] = t
        lngc = consts.tile([128, 4], F32, name="lngc")   # ln_g * sqrt(512)
        nc.scalar.dma_start(out=lngc, in_=lngd.rearrange("(c p) -> p c", p=128))
        lnbc = consts.tile([128, 4], F32, name="lnbc")
        nc.scalar.dma_start(out=lnbc, in_=lnbd.rearrange("(c p) -> p c", p=128))
        bc1c = consts.tile([128, 4], F32, name="bc1c")
        nc.scalar.dma_start(out=bc1c, in_=bc1d.rearrange("(c p) -> p c", p=128))
        bocbc = consts.tile([128, 4], F32, name="bocbc")
        nc.scalar.dma_start(out=bocbc, in_=bocbd.rearrange("(c p) -> p c", p=128))

        for sb in range(n_super):
            b0 = sb * SBATCH
            # ---- input DMAs (SP queue) ----
            xt8 = iop.tile([128, 4, SBATCH, 256], F8, name="xt8")
            ct8 = iop.tile([128, 4, SBATCH, 256], F8, name="ct8")
            ctb = iop.tile([128, 4, SBATCH, 256], BF16, name="ctb")
            for j in range(SBATCH):
                nc.sync.dma_start(
                    out=xt8[:, :, j, :],
                    in_=x8d[b0 + j].rearrange("(kc p) n -> p kc n", p=128))
                nc.sync.dma_start(
                    out=ct8[:, :, j, :],
                    in_=c8d[b0 + j].rearrange("(kc p) n -> p kc n", p=128))
                nc.sync.dma_start(
                    out=ctb[:, :, j, :],
                    in_=cbd[b0 + j].rearrange("(kc p) n -> p kc n", p=128))

            # ---- projections: fp8 DoubleRow ----
            qt = work.tile([128, 4, SBATCH, 256], BF16, name="qt")
            kt = work.tile([128, 4, SBATCH, 256], BF16, name="kt")
            for c in range(4):
                ps = psum.tile([128, NW], F32, tag="g", bufs=2)
                for i in range(2):
                    mm(ps, w8["wq"][:, 2 * i:2 * i + 2, c * 128:(c + 1) * 128],
                       xt8[:, 2 * i:2 * i + 2], start=i == 0, stop=i == 1,
                       perf_mode=DR)
                nc.scalar.copy(
                    out=qt[:, c].rearrange("p j n -> p (j n)"), in_=ps)
            for c in range(4):
                ps = psum.tile([128, NW], F32, tag="g", bufs=2)
                for i in range(2):
                    mm(ps, w8["wk"][:, 2 * i:2 * i + 2, c * 128:(c + 1) * 128],
                       ct8[:, 2 * i:2 * i + 2], start=i == 0, stop=i == 1,
                       perf_mode=DR)
                nc.vector.tensor_copy(
                    out=kt[:, c].rearrange("p j n -> p (j n)"), in_=ps)
            # v token-major with per-head denominator column
            vt = work.tile([128, SBATCH, 2, 8, 65], BF16, name="vt")
            nc.vector.memset(vt[:, :, :, :, 64:65], 2.0 ** (kv - OT_K))
            for j in range(SBATCH):
                for mc in range(2):
                    ps = psum.tile([128, 512], F32, tag="g", bufs=2)
                    for i in range(2):
                        mm(ps, ct8[:, 2 * i:2 * i + 2, j, mc * 128:(mc + 1) * 128],
                           w8["wv"][:, 2 * i:2 * i + 2], start=i == 0, stop=i == 1,
                           perf_mode=DR)
                    nc.vector.tensor_copy(
                        out=vt[:, j, mc, :, 0:64],
                        in_=ps.rearrange("p (h d) -> p h d", h=8))

            # ---- context branch: h = c @ Wc1 + bc1 (bf16, feature-major) ----
            ht = work.tile([128, 4, NW], BF16, name="ht")
            sqt = work.tile([128, 4, NW], BF16, name="sqt")
            for c in range(4):
                ps = psum.tile([128, NW], F32, tag="g", bufs=2)
                for kc in range(4):
                    mm(ps, wb["wc1"][:, kc, c * 128:(c + 1) * 128],
                       ctb[:, kc].rearrange("p j n -> p (j n)"),
                       start=kc == 0, stop=kc == 3)
                nc.scalar.activation(out=ht[:, c], in_=ps, func=AF.Identity,
                                     bias=bc1c[:, c:c + 1])
                nc.gpsimd.tensor_mul(out=sqt[:, c], in0=ht[:, c], in1=ht[:, c])

            # ---- LN stats ----
            mu_ps = psum.tile([128, NW], F32, tag="g", bufs=2)
            sq_ps = psum.tile([128, NW], F32, tag="g", bufs=2)
            for c in range(4):
                mm(mu_ps[0:1, :], colones, ht[:, c], start=c == 0, stop=c == 3)
            for c in range(4):
                mm(sq_ps[0:1, :], colones, sqt[:, c], start=c == 0, stop=c == 3)
            # 512*var = sq_sum - mu_sum^2/512
            mu_r = rows.tile([1, NW], F32, tag="r", bufs=6)
            nc.scalar.copy(out=mu_r, in_=mu_ps[0:1, :])
            ms_r = rows.tile([1, NW], F32, tag="r", bufs=6)
            nc.vector.scalar_tensor_tensor(
                out=ms_r, in0=mu_r, scalar=1.0 / DIM,
                in1=mu_r, op0=ALU.mult, op1=ALU.mult)
            var_r = rows.tile([1, NW], F32, tag="r", bufs=6)
            nc.vector.tensor_sub(out=var_r, in0=sq_ps[0:1, :], in1=ms_r)
            # a = rstd/sqrt(512) = exp(-0.5*ln(512*var + 512*eps))
            ln_r = rows.tile([1, NW], F32, tag="r", bufs=6)
            nc.scalar.activation(out=ln_r, in_=var_r, func=AF.Ln, bias=eps512)
            a_r = rows.tile([1, NW], BF16, tag="r", bufs=6)
            nc.scalar.activation(out=a_r, in_=ln_r, func=AF.Exp, scale=-0.5)
            # d = -(mu_sum/512) * a
            d_r = rows.tile([1, NW], BF16, tag="r", bufs=6)
            nc.vector.scalar_tensor_tensor(
                out=d_r, in0=mu_r, scalar=-1.0 / DIM,
                in1=a_r, op0=ALU.mult, op1=ALU.mult)
            # broadcast a,d to all 128 partitions (one 2-bank PSUM tile)
            ad_ps = psum.tile([128, 2, NW], F32, tag="s", bufs=2)
            mm(ad_ps[:, 0, :], onecol, a_r, start=True, stop=True)
            mm(ad_ps[:, 1, :], onecol, d_r, start=True, stop=True)
            ad_sb = work.tile([128, 2, NW], BF16, name="ad_sb")
            nc.vector.tensor_copy(out=ad_sb, in_=ad_ps)

            # ---- attention + interleaved LN-normalize / ctx2+out1 ----
            ot = work.tile([128, 4, SBATCH, 256], F8, name="ot")
            res = iop.tile([128, 4, SBATCH, 256], F32, name="res")

            def attn(c, j):
                s_ps = psum.tile([128, 2, NW], F32, tag="s", bufs=2)
                for h2 in range(2):
                    p0 = 64 * h2
                    for mc in range(2):
                        mm(s_ps[:, h2, mc * 256:(mc + 1) * 256],
                           kt[p0:p0 + 64, c, j, mc * 128:(mc + 1) * 128],
                           qt[p0:p0 + 64, c, j], start=True, stop=True)
                pt = ppool.tile([128, 2, 2, 256], BF16, tag="p", name="pt",
                                bufs=3)
                nc.scalar.activation(
                    out=pt.rearrange("p mc h2 n -> p h2 mc n"),
                    in_=s_ps.rearrange("p h2 (mc n) -> p h2 mc n", mc=2),
                    func=AF.Exp, scale=2.0 ** (-(kq + kk)))
                mbt = mbp.tile([128, 2, 2, 256], BF16, name="mbt")
                nc.sync.dma_start(
                    out=mbt,
                    in_=mbd[b0 + j, c].rearrange("mc h2 p n -> p mc h2 n"))
                nc.vector.tensor_mul(out=pt, in0=pt, in1=mbt)
                oo = psum.tile([65, 2, 256], F32, tag="oo", bufs=2)
                for h2 in range(2):
                    for mc in range(2):
                        mm(oo[:, h2, :], vt[:, j, mc, 2 * c + h2, :],
                           pt[:, mc, h2, :], start=mc == 0, stop=mc == 1)
                rec_r = rows.tile([1, NW], F32, tag="rec", bufs=2)
                nc.vector.reciprocal(
                    out=rec_r, in_=oo[64:65].rearrange("o h n -> o (h n)"))
                rec_bc = ppool.tile([128, NW], F32, tag="rb", name="rec_bc",
                                    bufs=2)
                nc.gpsimd.partition_broadcast(rec_bc, rec_r)
                for h2 in range(2):
                    nc.vector.tensor_mul(
                        out=ot[h2 * 64:(h2 + 1) * 64, c, j],
                        in0=oo[0:64, h2, :],
                        in1=rec_bc[h2 * 64:(h2 + 1) * 64,
                                   h2 * 256:(h2 + 1) * 256])

            def normalize(c):
                # rl = relu(((h*a + d)) * (g*sqrt(512)) + b), written in place
                nc.gpsimd.tensor_mul(out=ht[:, c], in0=ht[:, c],
                                     in1=ad_sb[:, 0, :])
                nc.gpsimd.tensor_add(out=ht[:, c], in0=ht[:, c],
                                     in1=ad_sb[:, 1, :])
                nc.scalar.activation(out=ht[:, c], in_=ht[:, c], func=AF.Relu,
                                     scale=lngc[:, c:c + 1],
                                     bias=lnbc[:, c:c + 1])

            def ctx2wo(j):
                co = psum.tile([128, 2, NW], F32, tag="s", bufs=2)
                for f in range(4):
                    dst = co[:, f // 2, (f % 2) * 256:(f % 2) * 256 + 256]
                    for kc in range(4):
                        mm(dst, wb["wc2"][:, kc, f * 128:(f + 1) * 128],
                           ht[:, kc, j * 256:(j + 1) * 256],
                           start=kc == 0, stop=False)
                    for i in range(2):
                        mm(dst,
                           w8["wo"][:, 2 * i:2 * i + 2, f * 128:(f + 1) * 128],
                           ot[:, 2 * i:2 * i + 2, j, :],
                           start=False, stop=i == 1, perf_mode=DR)
                for f in range(4):
                    nc.scalar.activation(
                        out=res[:, f, j, :],
                        in_=co[:, f // 2, (f % 2) * 256:(f % 2) * 256 + 256],
                        func=AF.Identity, scale=2.0 ** (-g),
                        bias=bocbc[:, f:f + 1])
                nc.gpsimd.dma_start(
                    out=outT[b0 + j].rearrange("(c p) n -> p c n", p=128),
                    in_=res[:, :, j, :])

            for j in range(SBATCH):
                for c in range(4):
                    attn(c, j)
                    if j == 0:
                        normalize(c)
                ctx2wo(j)


def build(n_super, ks):
    nc = bacc.Bacc("TRN2", target_bir_lowering=False, debug=False,
                   num_devices=N_CORES)
    dt = nc.dram_tensor
    io = (
        dt("x8", [BPC, DIM, N], F8, kind="ExternalInput").ap(),
        dt("c8", [BPC, DIM, N], F8, kind="ExternalInput").ap(),
        dt("cb", [BPC, DIM, N], BF16, kind="ExternalInput").ap(),
        dt("mb", [BPC, 4, 2, 2, 128, N], BF16, kind="ExternalInput").ap(),
        dt("wq", [DIM, DIM], F8, kind="ExternalInput").ap(),
        dt("wk", [DIM, DIM], F8, kind="ExternalInput").ap(),
        dt("wv", [DIM, DIM], F8, kind="ExternalInput").ap(),
        dt("wo", [DIM, DIM], F8, kind="ExternalInput").ap(),
        dt("wc1", [DIM, DIM], BF16, kind="ExternalInput").ap(),
        dt("wc2", [DIM, DIM], BF16, kind="ExternalInput").ap(),
        dt("lng", [DIM], F32, kind="ExternalInput").ap(),
        dt("lnb", [DIM], F32, kind="ExternalInput").ap(),
        dt("bc1", [DIM], F32, kind="ExternalInput").ap(),
        dt("bocb", [DIM], F32, kind="ExternalInput").ap(),
        dt("outT", [BPC, DIM, N], F32, kind="ExternalOutput").ap(),
    )
    with tile.TileContext(nc) as tc:
        _emit(nc, tc, io, n_super, ks)
    nc.compile()
    return nc


def _k_of(absmax):
    return int(math.floor(math.log2(120.0 / max(absmax, 1e-30))))


def prep_in_maps(x, context, mask, Wq, Wk, Wv, Wc1, bc1, ln_g, ln_b, Wc2, bc2,
                 Wo, bo, bias_table, rel_index):
    f = np.float32
    x = np.asarray(x, f)
    context = np.asarray(context, f)
    mask = np.asarray(mask)
    Wq = np.asarray(Wq, f) * SCALE
    Wk = np.asarray(Wk, f)
    Wv = np.asarray(Wv, f)
    Wo = np.asarray(Wo, f)
    Wc1 = np.asarray(Wc1, f)
    Wc2 = np.asarray(Wc2, f)

    kq = _k_of(np.abs(Wq).max())
    kk = _k_of(np.abs(Wk).max())
    kv = _k_of(np.abs(Wv).max())
    kwo = _k_of(np.abs(Wo).max())
    ks = (kq, kk, kv, kwo)
    g = OT_K + kwo

    xT = np.ascontiguousarray(
        x.reshape(N_CORES, BPC, N, DIM).transpose(0, 1, 3, 2))
    cT = np.ascontiguousarray(
        context.reshape(N_CORES, BPC, N, DIM).transpose(0, 1, 3, 2))
    x8 = xT.astype(NP8)
    c8 = cT.astype(NP8)
    cb = cT.astype(NPBF)

    # mb[core, b, c, mc, h2, p, n] = maskT[b, m, n] * exp(bias)[h, m, n]
    expBT = np.exp(
        np.asarray(bias_table, f)[np.asarray(rel_index)].transpose(2, 1, 0))
    # expBT: [H, m, n]; maskT: [core, b, m, n]
    mT = mask.reshape(N_CORES, BPC, N, N).transpose(0, 1, 3, 2).astype(f)
    mbf = mT[:, :, None, :, :] * expBT[None, None, :, :, :]  # [cr,b,h,m,n]
    mbf = mbf.reshape(N_CORES, BPC, 4, 2, 2, 128, N).transpose(
        0, 1, 2, 4, 3, 5, 6)  # [cr, b, c, mc, h2, p, n]
    mb = np.ascontiguousarray(mbf).astype(NPBF)

    shared = dict(
        wq=np.ascontiguousarray(Wq * 2.0 ** kq).astype(NP8),
        wk=np.ascontiguousarray(Wk * 2.0 ** kk).astype(NP8),
        wv=np.ascontiguousarray(Wv * 2.0 ** kv).astype(NP8),
        wo=np.ascontiguousarray(Wo * 2.0 ** kwo).astype(NP8),
        wc1=np.ascontiguousarray(Wc1).astype(NPBF),
        wc2=np.ascontiguousarray(Wc2 * 2.0 ** g).astype(NPBF),
        lng=np.ascontiguousarray(np.asarray(ln_g, f) * math.sqrt(DIM)),
        lnb=np.ascontiguousarray(np.asarray(ln_b, f)),
        bc1=np.ascontiguousarray(np.asarray(bc1, f)),
        bocb=np.ascontiguousarray(np.asarray(bo, f) + np.asarray(bc2, f)),
    )
    in_maps = [dict(x8=x8[c], c8=c8[c], cb=cb[c], mb=mb[c], **shared)
               for c in range(N_CORES)]
    return in_maps, ks


_nc_cache = {}


def _get_nc(n_super, ks):
    key = (n_super, ks)
    if key not in _nc_cache:
        _nc_cache[key] = build(n_super, ks)
    return _nc_cache[key]


def assemble_out(results):
    outT = np.stack([results[c]["outT"] for c in range(N_CORES)])
    return np.ascontiguousarray(
        outT.transpose(0, 1, 3, 2).reshape(B, N, DIM)).astype(np.float32)


def kernel(**inputs):
    in_maps, ks = prep_in_maps(**inputs)
    nc = _get_nc(NSUPER, ks)
    res = run_bass_kernel_spmd(nc, in_maps, core_ids=list(range(N_CORES)))
    return assemble_out(res.results)
